# revision 1
# baseline (speedup 1.0000x reference)
import sys
import numpy as np

for _p in ("/opt/trn_rl_repo", "/root/.axon_site/_ro/trn_rl_repo"):
    if _p not in sys.path:
        sys.path.append(_p)

B, N, NODE, FE = 128, 100, 2, 128
NODE_SIZES = [2, 16, 32]
FN2_OUT = [14, 30, 1]
NCORES = 8
GPC = B // NCORES            # graphs per core = 16
COLS = GPC * N               # 1600
PAIRS = N * N                # 10000
CH = 500                     # matmul moving chunk (<=512 fp32)
IPG = [5, 5, 5, 5, 5, 5, 5, 5, 5, 5, 5, 5, 5, 5, 5, 5, 5, 5, 5, 5]  # i's per chunk


def round_fp32r(a):
    u = np.ascontiguousarray(np.asarray(a, np.float32)).view(np.uint32)
    low = u & np.uint32(0xFFF)
    base = u & np.uint32(0xFFFFF000)
    add = ((low > 0x800) | ((low == 0x800) & (((u >> 12) & 1) == 1))).astype(np.uint32) << 12
    return (base + add).view(np.float32)


_CACHE = {}


def _build():
    import concourse.bacc as bacc
    import concourse.mybir as mybir
    import concourse.tile as tile

    F32 = mybir.dt.float32
    F32R = mybir.dt.float32r
    AF = mybir.ActivationFunctionType
    ALU = mybir.AluOpType
    AX = mybir.AxisListType

    nc = bacc.Bacc("TRN2", target_bir_lowering=False, debug=False, num_devices=NCORES)

    din = {}

    def I(name, shape, dt=F32R):
        din[name] = nc.dram_tensor(name, shape, dt, kind="ExternalInput")

    I("xt0", [2, COLS])
    I("nrmh", [GPC, PAIRS])
    for i in range(3):
        d = NODE_SIZES[i]
        od = FN2_OUT[i]
        I(f"w1a{i}", [d, 128])
        I(f"w1b{i}", [d, 128])
        I(f"w1c{i}", [1, 128])
        I(f"w1cb{i}", [2, 128])
        I(f"fe2T{i}", [128, 128])
        I(f"fn1avT{i}", [128, 128], F32R)
        I(f"fn1xT{i}", [d, 128])
        I(f"fn2T{i}", [128, od], F32R)
        I(f"b1_{i}", [128, 1], F32)
        I(f"b2_{i}", [128, 1], F32)
        I(f"fb1_{i}", [128, 1], F32)
        I(f"fb2_{i}", [od, 1], F32)
    out_d = nc.dram_tensor("out", [1, GPC], F32, kind="ExternalOutput")

    # chunk grouping: groups of 2 chunks (1000 cols) -> PSUM tile 2 banks
    GRP_CH = [(c, 2) for c in range(0, 20, 2)]

    with tile.TileContext(nc) as tc:
        with (
            tc.tile_pool(name="const", bufs=1) as cpool,
            tc.tile_pool(name="xp", bufs=1) as xpool,
            tc.tile_pool(name="wk", bufs=4) as wpool,
            tc.tile_pool(name="wk2", bufs=4) as wpool2,
            tc.tile_pool(name="stg", bufs=2) as spool,
            tc.tile_pool(name="sm", bufs=2) as smpool,
            tc.tile_pool(name="ps1", bufs=3, space="PSUM") as ps1,
            tc.tile_pool(name="ps2", bufs=1, space="PSUM") as ps2,
        ):
            W = {}
            for name, dt_ in din.items():
                if name == "nrmh":
                    continue
                sh = list(dt_.shape)
                W[name] = cpool.tile(sh, dt_.dtype, tag=name, name=f"w_{name}")
                nc.sync.dma_start(W[name][:], dt_.ap())

            xt0 = W["xt0"]

            # ---------- blocks ----------
            xt = xt0
            ty = None
            for blk in range(3):
                d = NODE_SIZES[blk]
                od = FN2_OUT[blk]
                w1a, w1b, w1c = W[f"w1a{blk}"], W[f"w1b{blk}"], W[f"w1c{blk}"]
                fe2T = W[f"fe2T{blk}"]
                b1, b2 = W[f"b1_{blk}"], W[f"b2_{blk}"]
                AV = xpool.tile([128, COLS], F32R, tag=f"av{blk}")

                for g in range(GPC):
                    gsl = slice(g * N, (g + 1) * N)
                    stg = spool.tile([1, PAIRS], F32R, tag="stg")
                    nc.sync.dma_start(stg[0:1, :], din["nrmh"].ap()[g:g + 1, :])

                    for (c0, nch) in GRP_CH:
                        span = nch * CH
                        p1 = ps1.tile([128, 1024], F32, tag="p1")
                        # fe1 = x1 part + x2 part + norm part (accumulate in PSUM)
                        # chunk ci lives at bank-aligned offset ci*512, cols [0,500)
                        for ci in range(nch):
                            i0 = (c0 + ci) * 5
                            r1 = xt[:, g * N + i0: g * N + i0 + 5].unsqueeze(2).broadcast_to([d, 5, N])
                            nc.tensor.matmul(p1[:, ci * 512: ci * 512 + CH], w1a[:], r1,
                                             start=True, stop=False)
                        for ci in range(nch):
                            r2 = xt[:, gsl].unsqueeze(1).broadcast_to([d, 5, N])
                            nc.tensor.matmul(p1[:, ci * 512: ci * 512 + CH], w1b[:], r2,
                                             start=False, stop=False)
                        gi = c0 // 2
                        tg = g * 10 + gi
                        on_dve = False
                        for ci in range(nch):
                            cc = (c0 + ci) * CH
                            nc.tensor.matmul(p1[:, ci * 512: ci * 512 + CH], w1c[:],
                                             stg[0:1, cc:cc + CH], start=False, stop=True)
                        # Lrelu (exact, alpha=0.2) -> t1
                        p1v = p1[:, 0:nch * 512].rearrange("p (a b) -> p a b", b=512)[:, :, 0:CH]
                        t1 = wpool.tile([128, 1000], F32R, tag="t1")
                        t1v = t1[:, 0:span].rearrange("p (a b) -> p a b", b=CH)
                        if on_dve:
                            # v = z + b1; lrelu = max(v, 0.2v), b1 via scalar slots
                            u = smpool.tile([128, 1000], F32, tag="u_dve")
                            uv = u[:, 0:span].rearrange("p (a b) -> p a b", b=CH)
                            nc.vector.tensor_scalar(uv, p1v, b1[:], 0.2, ALU.add, ALU.mult)
                            nc.vector.scalar_tensor_tensor(
                                t1v, p1v, b1[:], uv, ALU.add, ALU.max)
                        else:
                            nc.scalar.activation(t1v, p1v, AF.Prelu,
                                                 bias=b1[:], scale=1.0, alpha=0.2)
                        # fe2
                        p2 = ps2.tile([128, 1024], F32, tag="p2")
                        for ci in range(nch):
                            nc.tensor.matmul(p2[:, ci * 512: ci * 512 + CH], fe2T[:],
                                             t1[:, ci * CH:(ci + 1) * CH], start=True, stop=True)
                        p2v = p2[:, 0:nch * 512].rearrange("p (a b) -> p a b", b=512)[:, :, 0:CH]
                        av2 = wpool2.tile([128, 1000], F32, tag="av2")
                        av2v = av2[:, 0:span].rearrange("p (a b) -> p a b", b=CH)
                        nc.scalar.activation(av2v, p2v, AF.Prelu,
                                             bias=b2[:], scale=1.0, alpha=0.2)
                        # sum over j within each i (segments of 100)
                        n_i = span // N
                        a2r = av2[:, 0:span].rearrange("p (a b) -> p a b", a=n_i)
                        if False:
                            trh = smpool.tile([128, 500], F32, tag="trh")
                            nc.gpsimd.tensor_tensor(
                                trh[:].rearrange("p (a b) -> p a b", b=50),
                                a2r[:, :, 0:50], a2r[:, :, 50:100], ALU.add)
                            rsrc = trh[:].rearrange("p (a b) -> p a b", a=n_i)
                        else:
                            rsrc = a2r
                        with nc.allow_low_precision(reason="f32r av"):
                            nc.vector.tensor_reduce(
                                AV[:, g * N + c0 * 5: g * N + c0 * 5 + n_i],
                                rsrc, axis=AX.X, op=ALU.add,
                            )

                # ----- fn stage over all COLS -----
                fn1avT, fn1xT = W[f"fn1avT{blk}"], W[f"fn1xT{blk}"]
                fn2T = W[f"fn2T{blk}"]
                fb1, fb2 = W[f"fb1_{blk}"], W[f"fb2_{blk}"]
                if blk < 2:
                    nd = NODE_SIZES[blk + 1]
                    xnext = xpool.tile([nd, COLS], F32R, tag=f"x{blk + 1}")
                else:
                    ty = xpool.tile([1, COLS], F32, tag="ty")
                for fc in range(4):
                    csl = slice(fc * 400, (fc + 1) * 400)
                    pf = ps1.tile([128, 400], F32, tag="p1")
                    nc.tensor.matmul(pf[:], fn1avT[:], AV[:, csl], start=True, stop=False)
                    nc.tensor.matmul(pf[:], fn1xT[:], xt[:, csl], start=False, stop=True)
                    y1 = wpool.tile([128, 400], F32R, tag="y1")
                    nc.scalar.activation(y1[:], pf[:], AF.Tanh, bias=fb1[:])
                    pf2 = ps2.tile([od, 400], F32, tag="p2")
                    nc.tensor.matmul(pf2[:], fn2T[:], y1[:], start=True, stop=True)
                    if blk < 2:
                        nc.scalar.activation(xnext[0:od, csl], pf2[:], AF.Tanh, bias=fb2[:])
                    else:
                        nc.scalar.activation(ty[:, csl], pf2[:], AF.Tanh, bias=fb2[:])
                if blk < 2:
                    nc.sync.dma_start(xnext[od:od + 2, :], xt0[:, :])
                    xt = xnext

            # ---------- final: sigmoid(mean over N) ----------
            red = xpool.tile([1, GPC], F32, tag="red")
            nc.vector.tensor_reduce(red[:], ty[:].rearrange("p (a b) -> p a b", a=GPC),
                                    axis=AX.X, op=ALU.add)
            osb = xpool.tile([1, GPC], F32, tag="osb")
            nc.scalar.activation(osb[:], red[:], AF.Sigmoid, scale=1.0 / N)
            nc.sync.dma_start(out_d.ap(), osb[:])

    nc.compile()
    return nc


def _host_prep(inputs):
    """Build per-core in_maps from full inputs."""
    x = np.asarray(inputs["x"], np.float32)          # [B, N, 2]
    shared = {}
    for i in range(3):
        d = NODE_SIZES[i]
        fe1w = np.asarray(inputs[f"fe1w{i}"], np.float32)   # [128, 2d+1]
        fe1b = np.asarray(inputs[f"fe1b{i}"], np.float32)
        fe2w = np.asarray(inputs[f"fe2w{i}"], np.float32)   # [128, 128]
        fe2b = np.asarray(inputs[f"fe2b{i}"], np.float32)
        fn1w = np.asarray(inputs[f"fn1w{i}"], np.float32)   # [128, 128+d]
        fn1b = np.asarray(inputs[f"fn1b{i}"], np.float32)
        fn2w = np.asarray(inputs[f"fn2w{i}"], np.float32)   # [od, 128]
        fn2b = np.asarray(inputs[f"fn2b{i}"], np.float32)
        if i == 0:
            perm = np.arange(d)
        else:
            # my x row order [y..., c0, c1] -> ref order [c0, c1, y...]
            perm = np.concatenate([np.arange(2, d), [0, 1]])
        W1a = fe1w[:, 0:d][:, perm]
        W1b = fe1w[:, d:2 * d][:, perm]
        w1c = fe1w[:, 2 * d]
        shared[f"w1a{i}"] = round_fp32r(np.ascontiguousarray(W1a.T))
        shared[f"w1b{i}"] = round_fp32r(np.ascontiguousarray(W1b.T))
        shared[f"w1c{i}"] = round_fp32r(w1c.reshape(1, 128))
        shared[f"w1cb{i}"] = round_fp32r(np.stack([w1c, fe1b]))
        shared[f"fe2T{i}"] = round_fp32r(np.ascontiguousarray(fe2w.T))
        shared[f"fn1avT{i}"] = round_fp32r(np.ascontiguousarray(fn1w[:, :128].T))
        shared[f"fn1xT{i}"] = round_fp32r(np.ascontiguousarray(fn1w[:, 128:][:, perm].T))
        shared[f"fn2T{i}"] = round_fp32r(np.ascontiguousarray(fn2w.T))
        shared[f"b1_{i}"] = fe1b.reshape(128, 1)
        shared[f"b2_{i}"] = fe2b.reshape(128, 1)
        shared[f"fb1_{i}"] = fn1b.reshape(128, 1)
        shared[f"fb2_{i}"] = fn2b.reshape(FN2_OUT[i], 1)

    in_maps = []
    for c in range(NCORES):
        xf = x[c * GPC:(c + 1) * GPC]                        # [16, 100, 2] full precision
        xs = round_fp32r(xf)
        xt0 = np.ascontiguousarray(xs.transpose(2, 0, 1).reshape(2, COLS))
        diff = xf[:, :, None, :] - xf[:, None, :, :]
        nrm = np.sqrt((diff * diff).sum(-1)).reshape(GPC, PAIRS)
        m = dict(shared)
        m["xt0"] = xt0
        m["nrmh"] = round_fp32r(nrm)
        in_maps.append(m)
    return in_maps


def kernel(**inputs):
    from concourse import bass_utils

    if "nc" not in _CACHE:
        _CACHE["nc"] = _build()
    nc = _CACHE["nc"]
    in_maps = _host_prep(inputs)
    res = bass_utils.run_bass_kernel_spmd(nc, in_maps, core_ids=list(range(NCORES)))
    out = np.concatenate(
        [np.asarray(res.results[c]["out"], np.float32).reshape(GPC, 1) for c in range(NCORES)],
        axis=0,
    )
    return out



# revision 2
# speedup vs baseline: 1.2072x; 1.2072x over previous
import sys
import numpy as np

for _p in ("/opt/trn_rl_repo", "/root/.axon_site/_ro/trn_rl_repo"):
    if _p not in sys.path:
        sys.path.append(_p)

import ml_dtypes

BF = ml_dtypes.bfloat16

B, N, NODE, FE = 128, 100, 2, 128
NODE_SIZES = [2, 16, 32]
FN2_OUT = [14, 30, 1]
NCORES = 8
GPC = B // NCORES            # graphs per core = 16
COLS = GPC * N               # 1600
PAIRS = N * N                # 10000
UPG = 10                     # units (1000-col) per graph
UNITS = GPC * UPG            # 160 units per block

# Per-unit engine assignment for the t1 leaky-relu (consumes fe1 PSUM):
# 'P' = gpsimd/Pool stt, 'D' = vector/DVE stt, 'A' = scalar/ACT Prelu.
T1_PAT = "PPDPPPAPPD"
# Reduce level-1 adds: 'D' = DVE, 'P' = Pool
L1_PAT = "D"


def round_fp32r(a):
    u = np.ascontiguousarray(np.asarray(a, np.float32)).view(np.uint32)
    low = u & np.uint32(0xFFF)
    base = u & np.uint32(0xFFFFF000)
    add = ((low > 0x800) | ((low == 0x800) & (((u >> 12) & 1) == 1))).astype(np.uint32) << 12
    return (base + add).view(np.float32)


_CACHE = {}


def _build():
    import concourse.bacc as bacc
    import concourse.mybir as mybir
    import concourse.tile as tile

    F32 = mybir.dt.float32
    F32R = mybir.dt.float32r
    BF16 = mybir.dt.bfloat16
    AF = mybir.ActivationFunctionType
    ALU = mybir.AluOpType
    AX = mybir.AxisListType

    nc = bacc.Bacc("TRN2", target_bir_lowering=False, debug=False, num_devices=NCORES)

    din = {}

    def I(name, shape, dt):
        din[name] = nc.dram_tensor(name, shape, dt, kind="ExternalInput")

    I("xt0", [3, COLS], BF16)            # rows [c0; c1; ones]
    I("nrmo", [2 * GPC, PAIRS], BF16)    # rows 2g = nrm(g), 2g+1 = ones
    for i in range(3):
        d = NODE_SIZES[i]
        od = FN2_OUT[i]
        I(f"s1_{i}", [2 * d + 2, 128], BF16)   # [W1a-perm; W1b-perm; w1c; b1]
        I(f"fe2T{i}", [128, 128], BF16)
        I(f"b2_{i}", [128, 1], F32)
        I(f"fn1avT{i}", [128, 128], F32R)
        I(f"fn1xT{i}", [d, 128], BF16)
        I(f"fn2T{i}", [128, od], F32R)
        I(f"fb1_{i}", [128, 1], F32)
        I(f"fb2_{i}", [od, 1], F32)
    out_d = nc.dram_tensor("out", [1, GPC], F32, kind="ExternalOutput")

    with tile.TileContext(nc) as tc:
        with (
            tc.tile_pool(name="const", bufs=1) as cpool,
            tc.tile_pool(name="xp", bufs=1) as xpool,
            tc.tile_pool(name="m2", bufs=2) as mpool,
            tc.tile_pool(name="rep", bufs=2) as rpool,
            tc.tile_pool(name="t1p", bufs=4) as tpool,
            tc.tile_pool(name="avp", bufs=4) as apool,
            tc.tile_pool(name="q1p", bufs=3) as q1pool,
            tc.tile_pool(name="q2p", bufs=3) as q2pool,
            tc.tile_pool(name="y1p", bufs=2) as y1pool,
            tc.tile_pool(name="ps1", bufs=2, space="PSUM") as ps1,
            tc.tile_pool(name="ps2", bufs=2, space="PSUM") as ps2,
        ):
            W = {}
            for name, dt_ in din.items():
                if name == "nrmo":
                    continue
                sh = list(dt_.shape)
                W[name] = cpool.tile(sh, dt_.dtype, tag=name, name=f"w_{name}")
                nc.sync.dma_start(W[name][:], dt_.ap())

            xt = W["xt0"]                      # [3, COLS] block-0 x (+ones)
            ty = None
            for blk in range(3):
                d = NODE_SIZES[blk]
                od = FN2_OUT[blk]
                s1 = W[f"s1_{blk}"]
                fe2T = W[f"fe2T{blk}"]
                b2 = W[f"b2_{blk}"]
                AV = xpool.tile([128, COLS], F32R, tag=f"av{blk}")

                for g in range(GPC):
                    gsl = slice(g * N, (g + 1) * N)
                    # ---- build M2 = [x1; x2; nrm; ones]  [2d+2, PAIRS] bf16
                    rep4 = rpool.tile([d, 400], BF16, tag="rep4")
                    nc.sync.dma_start(
                        rep4[:].rearrange("p (a b) -> p a b", a=4),
                        xt[0:d, gsl].unsqueeze(1).broadcast_to([d, 4, N]),
                    )
                    M2 = mpool.tile([2 * d + 2, PAIRS], BF16, tag="m2")
                    # x1 rows: element-broadcast (each x[k,i] repeated N times)
                    nc.sync.dma_start(
                        M2[0:d, :].rearrange("p (a b) -> p a b", b=N),
                        xt[0:d, gsl].unsqueeze(2).broadcast_to([d, N, N]),
                    )
                    # x2 rows: block-tiled (x row repeated 25x as 400-col runs)
                    nc.sync.dma_start(
                        M2[d:2 * d, :].rearrange("p (a b) -> p a b", a=25),
                        rep4[:].unsqueeze(1).broadcast_to([d, 25, 400]),
                    )
                    # nrm + ones rows straight from HBM
                    nc.sync.dma_start(
                        M2[2 * d:2 * d + 2, :],
                        din["nrmo"].ap()[2 * g:2 * g + 2, :],
                    )

                    for u in range(UPG):
                        k = g * UPG + u
                        # ---- fe1: one matmul pass (bias included via ones row)
                        p1 = ps1.tile([128, 1024], F32, tag="p1")
                        for ci in range(2):
                            c0 = u * 1000 + ci * 500
                            nc.tensor.matmul(p1[:, ci * 512: ci * 512 + 500],
                                             s1[:], M2[:, c0:c0 + 500],
                                             start=True, stop=True)
                        p1v = p1[:].rearrange("p (a b) -> p a b", b=512)[:, :, 0:500]
                        t1 = tpool.tile([128, 1000], BF16, tag="t1")
                        t1v = t1[:].rearrange("p (a b) -> p a b", b=500)
                        e = T1_PAT[k % len(T1_PAT)]
                        if e == "A":
                            nc.scalar.activation(t1v, p1v, AF.Prelu,
                                                 scale=1.0, alpha=0.2)
                        else:
                            eng = nc.vector if e == "D" else nc.gpsimd
                            eng.scalar_tensor_tensor(t1v, p1v, 0.2, p1v,
                                                     ALU.mult, ALU.max)
                        # ---- fe2
                        p2 = ps2.tile([128, 1024], F32, tag="p2")
                        for ci in range(2):
                            nc.tensor.matmul(p2[:, ci * 512: ci * 512 + 500],
                                             fe2T[:], t1[:, ci * 500:(ci + 1) * 500],
                                             start=True, stop=True)
                        p2v = p2[:].rearrange("p (a b) -> p a b", b=512)[:, :, 0:500]
                        av2 = apool.tile([128, 1000], BF16, tag="av2")
                        av2v = av2[:].rearrange("p (a b) -> p a b", b=500)
                        nc.scalar.activation(av2v, p2v, AF.Prelu,
                                             bias=b2[:], scale=1.0, alpha=0.2)
                        # ---- sum over j (100) per i: bf16 tree + reduce tail
                        a3 = av2[:].rearrange("p (a b) -> p a b", b=N)   # [128,10,100]
                        q1 = q1pool.tile([128, 500], BF16, tag="q1")
                        q1v = q1[:].rearrange("p (a b) -> p a b", b=50)
                        l1eng = nc.vector if L1_PAT[k % len(L1_PAT)] == "D" else nc.gpsimd
                        l1eng.tensor_tensor(q1v, a3[:, :, 0:50], a3[:, :, 50:100],
                                            ALU.add)
                        q2 = q2pool.tile([128, 250], BF16, tag="q2")
                        q2v = q2[:].rearrange("p (a b) -> p a b", b=25)
                        nc.vector.tensor_tensor(q2v, q1v[:, :, 0:25], q1v[:, :, 25:50],
                                                ALU.add)
                        with nc.allow_low_precision(reason="f32r av"):
                            nc.vector.tensor_reduce(
                                AV[:, g * N + u * UPG: g * N + u * UPG + UPG],
                                q2v, axis=AX.X, op=ALU.add,
                            )

                # ----- fn stage over all COLS -----
                fn1avT, fn1xT = W[f"fn1avT{blk}"], W[f"fn1xT{blk}"]
                fn2T = W[f"fn2T{blk}"]
                fb1, fb2 = W[f"fb1_{blk}"], W[f"fb2_{blk}"]
                if blk < 2:
                    nd = NODE_SIZES[blk + 1]
                    xnext = xpool.tile([nd + 1, COLS], BF16, tag=f"x{blk + 1}")
                else:
                    ty = xpool.tile([1, COLS], F32, tag="ty")
                for fc in range(4):
                    csl = slice(fc * 400, (fc + 1) * 400)
                    pf = ps1.tile([128, 400], F32, tag="p1")
                    nc.tensor.matmul(pf[:], fn1avT[:], AV[:, csl], start=True, stop=False)
                    nc.tensor.matmul(pf[:], fn1xT[:], xt[0:d, csl], start=False, stop=True)
                    y1 = y1pool.tile([128, 400], F32R, tag="y1")
                    nc.scalar.activation(y1[:], pf[:], AF.Tanh, bias=fb1[:])
                    pf2 = ps2.tile([od, 400], F32, tag="p2")
                    nc.tensor.matmul(pf2[:], fn2T[:], y1[:], start=True, stop=True)
                    if blk < 2:
                        nc.scalar.activation(xnext[0:od, csl], pf2[:], AF.Tanh, bias=fb2[:])
                    else:
                        nc.scalar.activation(ty[:, csl], pf2[:], AF.Tanh, bias=fb2[:])
                if blk < 2:
                    # coords + ones rows [c0; c1; ones] appended below the y rows
                    nc.sync.dma_start(xnext[od:od + 3, :], W["xt0"][:, :])
                    xt = xnext

            # ---------- final: sigmoid(mean over N) ----------
            red = xpool.tile([1, GPC], F32, tag="red")
            nc.vector.tensor_reduce(red[:], ty[:].rearrange("p (a b) -> p a b", a=GPC),
                                    axis=AX.X, op=ALU.add)
            osb = xpool.tile([1, GPC], F32, tag="osb")
            nc.scalar.activation(osb[:], red[:], AF.Sigmoid, scale=1.0 / N)
            nc.sync.dma_start(out_d.ap(), osb[:])

    nc.compile()
    return nc


def _host_prep(inputs):
    """Build per-core in_maps from full inputs."""
    x = np.asarray(inputs["x"], np.float32)          # [B, N, 2]
    shared = {}
    for i in range(3):
        d = NODE_SIZES[i]
        fe1w = np.asarray(inputs[f"fe1w{i}"], np.float32)   # [128, 2d+1]
        fe1b = np.asarray(inputs[f"fe1b{i}"], np.float32)
        fe2w = np.asarray(inputs[f"fe2w{i}"], np.float32)   # [128, 128]
        fe2b = np.asarray(inputs[f"fe2b{i}"], np.float32)
        fn1w = np.asarray(inputs[f"fn1w{i}"], np.float32)   # [128, 128+d]
        fn1b = np.asarray(inputs[f"fn1b{i}"], np.float32)
        fn2w = np.asarray(inputs[f"fn2w{i}"], np.float32)   # [od, 128]
        fn2b = np.asarray(inputs[f"fn2b{i}"], np.float32)
        if i == 0:
            perm = np.arange(d)
        else:
            # my x row order [y..., c0, c1] -> ref order [c0, c1, y...]
            perm = np.concatenate([np.arange(2, d), [0, 1]])
        W1a = fe1w[:, 0:d][:, perm].T                       # [d, 128]
        W1b = fe1w[:, d:2 * d][:, perm].T                   # [d, 128]
        s1 = np.concatenate(
            [W1a, W1b, fe1w[:, 2 * d].reshape(1, 128), fe1b.reshape(1, 128)], axis=0
        )
        shared[f"s1_{i}"] = np.ascontiguousarray(s1).astype(BF)
        shared[f"fe2T{i}"] = np.ascontiguousarray(fe2w.T).astype(BF)
        shared[f"b2_{i}"] = fe2b.reshape(128, 1)
        shared[f"fn1avT{i}"] = round_fp32r(np.ascontiguousarray(fn1w[:, :128].T))
        shared[f"fn1xT{i}"] = np.ascontiguousarray(fn1w[:, 128:][:, perm].T).astype(BF)
        shared[f"fn2T{i}"] = round_fp32r(np.ascontiguousarray(fn2w.T))
        shared[f"fb1_{i}"] = fn1b.reshape(128, 1)
        shared[f"fb2_{i}"] = fn2b.reshape(FN2_OUT[i], 1)

    in_maps = []
    for c in range(NCORES):
        xf = x[c * GPC:(c + 1) * GPC]                        # [16, 100, 2]
        xt0 = np.concatenate(
            [xf.transpose(2, 0, 1).reshape(2, COLS), np.ones((1, COLS), np.float32)],
            axis=0,
        )
        diff = xf[:, :, None, :] - xf[:, None, :, :]
        nrm = np.sqrt((diff * diff).sum(-1)).reshape(GPC, PAIRS)
        nrmo = np.empty((2 * GPC, PAIRS), np.float32)
        nrmo[0::2] = nrm
        nrmo[1::2] = 1.0
        m = dict(shared)
        m["xt0"] = xt0.astype(BF)
        m["nrmo"] = nrmo.astype(BF)
        in_maps.append(m)
    return in_maps


def kernel(**inputs):
    from concourse import bass_utils

    if "nc" not in _CACHE:
        _CACHE["nc"] = _build()
    nc = _CACHE["nc"]
    in_maps = _host_prep(inputs)
    res = bass_utils.run_bass_kernel_spmd(nc, in_maps, core_ids=list(range(NCORES)))
    out = np.concatenate(
        [np.asarray(res.results[c]["out"], np.float32).reshape(GPC, 1) for c in range(NCORES)],
        axis=0,
    )
    return out
